# revision 19
# baseline (speedup 1.0000x reference)
"""Trainium2 Bass kernel: causal transformer encoder layer (pre-QKV fused),
SPMD across 8 NeuronCores.

Sharding: core c handles batch b = c//2.  The two cores of a batch split the
2048 query positions into 4 chunks of 256 each, paired so both halves get the
same total causal work AND the same static program structure (SPMD): chunk
slots have structure extents E = [16, 12, 8, 4] k-tiles (of 128); the two
halves' actual extents differ by exactly 2 at each slot, the difference is
absorbed by per-core mask DATA (multiplicative 0/1 masks on exp(scores)).

Layouts on device (all SBUF tiles are [128 partitions, ...]):
  activations feature-major [D, tokens] for matmul inputs,
  scores transposed S_T[k, q] so softmax-normalization denominators come from
  a fused ones-column in the AV stationary ([V | 1]), and the only
  partition-broadcast needed is inv-denominator -> 64 rows (gpsimd).
Matmuls run as float32r (full-rate fp32 on the PE for moving dim >= 256);
the attention probabilities and V are bf16.
"""

import os
from contextlib import ExitStack
from dataclasses import dataclass, field

import numpy as np
import ml_dtypes

import concourse.bass as bass
import concourse.bacc as bacc
import concourse.tile as tile
from concourse import mybir
from concourse.bass_utils import run_bass_kernel_spmd
from concourse.masks import make_identity
from concourse import library_config

F32 = mybir.dt.float32
F32R = mybir.dt.float32r
BF16 = mybir.dt.bfloat16
AF = mybir.ActivationFunctionType
ALU = mybir.AluOpType

EPS = 1e-5


@dataclass
class Cfg:
    B: int = 4
    S: int = 2048
    D: int = 512
    F: int = 2048
    H: int = 8
    CHUNK: int = 256
    KT: int = 128
    # which ops to emit (skip ops that are no-ops for the actual input values)
    use_bq: bool = False
    use_bk: bool = False
    use_bv: bool = False
    use_b1: bool = False
    use_b2: bool = False
    use_g1: bool = False
    use_bn1: bool = False
    use_g2: bool = False
    use_bn2: bool = False

    @property
    def HD(self):
        return self.D // self.H

    @property
    def DK(self):
        return self.D // 128  # number of 128-row tiles of D

    @property
    def FK(self):
        return self.F // 128

    @property
    def NCH(self):
        return self.S // self.CHUNK  # chunks per batch-sequence

    @property
    def NQ(self):
        return (self.NCH // 2) * self.CHUNK  # local query tokens per core

    @property
    def NSLOT(self):
        return self.NCH // 2

    @property
    def QT(self):
        return self.NQ // 128  # local q 128-tiles

    def ext(self, ci):
        return ((ci + 1) * self.CHUNK) // self.KT

    def slot_chunks(self, half):
        n = self.NCH
        if half == 0:
            s = [i for i in range(n) if i % 4 in (0, 3)]
        else:
            s = [i for i in range(n) if i % 4 in (1, 2)]
        return sorted(s, key=lambda ci: -self.ext(ci))

    def slot_qs(self, half):
        return [ci * self.CHUNK for ci in self.slot_chunks(half)]

    def slot_E(self):
        a = self.slot_chunks(0)
        b = self.slot_chunks(1)
        E = [max(self.ext(x), self.ext(y)) for x, y in zip(a, b)]
        for e in E:
            assert e % 4 == 0, E
        return E


def build_nc(cfg: Cfg) -> bass.Bass:
    S, D, F, H, HD = cfg.S, cfg.D, cfg.F, cfg.H, cfg.HD
    DK, FK, QT, NQ, CHUNK, KT = cfg.DK, cfg.FK, cfg.QT, cfg.NQ, cfg.CHUNK, cfg.KT
    NSLOT = cfg.NSLOT
    E = cfg.slot_E()
    HPT = 128 // HD  # heads per 128-row tile (2)
    WQ = min(512, NQ)   # moving width for q-token chunks
    WS = min(512, S)    # moving width for full-seq token chunks
    NSUB = WQ // 128

    nc = bacc.Bacc("TRN2", target_bir_lowering=False)

    xT_d = nc.declare_dram_parameter("xT", [D, S], BF16, isOutput=False)
    xqT_d = nc.declare_dram_parameter("xqT", [D, NQ], BF16, isOutput=False)
    xown_d = nc.declare_dram_parameter("xown", [NQ, D], F32, isOutput=False)
    wqkv_d = nc.declare_dram_parameter("wqkvT", [D, 3 * D], BF16, isOutput=False)
    wo_d = nc.declare_dram_parameter("woT", [D, D], BF16, isOutput=False)
    w1_d = nc.declare_dram_parameter("w1T", [D, F], BF16, isOutput=False)
    w2_d = nc.declare_dram_parameter("w2T", [F, D], BF16, isOutput=False)
    bq_d = nc.declare_dram_parameter("bq", [D], F32, isOutput=False)
    bk_d = nc.declare_dram_parameter("bk", [D], F32, isOutput=False)
    bv_d = nc.declare_dram_parameter("bv", [D], F32, isOutput=False)
    b1_d = nc.declare_dram_parameter("b1", [F], F32, isOutput=False)
    b2_d = nc.declare_dram_parameter("b2", [128, D], F32, isOutput=False)
    g1_d = nc.declare_dram_parameter("g1v", [128, D], F32, isOutput=False)
    bn1_d = nc.declare_dram_parameter("bn1v", [128, D], F32, isOutput=False)
    g2_d = nc.declare_dram_parameter("g2v", [128, D], F32, isOutput=False)
    bn2_d = nc.declare_dram_parameter("bn2v", [128, D], F32, isOutput=False)
    masks_d = nc.declare_dram_parameter(
        "masks", [128, NSLOT, 4 * CHUNK], BF16, isOutput=False
    )
    out_d = nc.declare_dram_parameter("out", [NQ, D], F32, isOutput=True)

    with ExitStack() as top:
        tc = top.enter_context(tile.TileContext(nc, pool_alloc_mode="queue"))
        consts = top.enter_context(tc.tile_pool(name="consts", bufs=1))

        ident = consts.tile([128, 128], F32)
        make_identity(nc, ident)

        masks_sb = consts.tile([128, NSLOT, 4 * CHUNK], BF16)
        nc.sync.dma_start(out=masks_sb, in_=masks_d[:, :, :])

        eps_sb = consts.tile([128, 1], F32)
        nc.vector.memset(eps_sb, EPS)

        bq_sb = bk_sb = bv_sb = b1_sb = None
        if cfg.use_bq:
            bq_sb = consts.tile([128, DK], F32)
            nc.sync.dma_start(out=bq_sb, in_=bq_d.rearrange("(m p) -> p m", p=128))
        if cfg.use_bk:
            bk_sb = consts.tile([128, DK], F32)
            nc.sync.dma_start(out=bk_sb, in_=bk_d.rearrange("(m p) -> p m", p=128))
        if cfg.use_bv:
            bv_sb = consts.tile([128, DK], F32)
            nc.sync.dma_start(out=bv_sb, in_=bv_d.rearrange("(m p) -> p m", p=128))
        if cfg.use_b1:
            b1_sb = consts.tile([128, FK], F32)
            nc.sync.dma_start(out=b1_sb, in_=b1_d.rearrange("(f p) -> p f", p=128))

        def bcast_const(dram, nm):
            t = consts.tile([128, D], F32, name=nm, tag=nm)
            nc.sync.dma_start(out=t, in_=dram[:, :])
            return t

        g1_b = bcast_const(g1_d, "g1b") if cfg.use_g1 else None
        bn1_b = bcast_const(bn1_d, "bn1b") if cfg.use_bn1 else None
        g2_b = bcast_const(g2_d, "g2b") if cfg.use_g2 else None
        bn2_b = bcast_const(bn2_d, "bn2b") if cfg.use_bn2 else None
        b2_b = bcast_const(b2_d, "b2b") if cfg.use_b2 else None

        # pools that outlive phase A open first (releases must be LIFO)
        pctx = top.enter_context(tc.tile_pool(name="pctx", bufs=1))
        ctx_fm = pctx.tile([128, DK, NQ], BF16)

        pqkv_cm = tc.tile_pool(name="pqkv", bufs=1)
        pqkv = pqkv_cm.__enter__()

        # ---------------- Phase A: QKV projections --------------------------
        pa_cm = tc.tile_pool(name="pa", bufs=1)
        pa = pa_cm.__enter__()

        xT_sb = pa.tile([128, DK, S], BF16)
        nc.sync.dma_start(out=xT_sb, in_=xT_d.rearrange("(m p) t -> p m t", p=128))
        xqT_sb = pa.tile([128, DK, NQ], BF16)
        nc.sync.dma_start(out=xqT_sb, in_=xqT_d.rearrange("(m p) t -> p m t", p=128))
        wqkv_sb = pa.tile([128, DK, 3 * D], BF16)
        nc.sync.dma_start(out=wqkv_sb, in_=wqkv_d.rearrange("(m p) c -> p m c", p=128))

        Qfm = pqkv.tile([128, DK, NQ], BF16)
        Kfm = pqkv.tile([128, DK, S], BF16)
        Vaug = pqkv.tile([128, S // KT, H * (HD + 1)], BF16)

        with tc.tile_pool(name="pa_psum", bufs=4, space="PSUM") as pap:
            # Q (own tokens, feature-major): lhsT = WqT tile, rhs = xqT
            for m in range(DK):
                for ch in range(NQ // WQ):
                    ps = pap.tile([128, WQ], F32, tag="ps")
                    for k in range(DK):
                        nc.tensor.matmul(
                            out=ps,
                            lhsT=wqkv_sb[:, k, m * 128 : (m + 1) * 128],
                            rhs=xqT_sb[:, k, ch * WQ : (ch + 1) * WQ],
                            start=(k == 0),
                            stop=(k == DK - 1),
                        )
                    dst = Qfm[:, m, ch * WQ : (ch + 1) * WQ]
                    if cfg.use_bq:
                        nc.scalar.activation(
                            out=dst, in_=ps, func=AF.Identity,
                            bias=bq_sb[:, m : m + 1], scale=1.0,
                        )
                    else:
                        nc.scalar.copy(dst, ps)
            # K (all tokens, feature-major)
            for m in range(DK):
                for ch in range(S // WS):
                    ps = pap.tile([128, WS], F32, tag="ps")
                    for k in range(DK):
                        nc.tensor.matmul(
                            out=ps,
                            lhsT=wqkv_sb[:, k, D + m * 128 : D + (m + 1) * 128],
                            rhs=xT_sb[:, k, ch * WS : (ch + 1) * WS],
                            start=(k == 0),
                            stop=(k == DK - 1),
                        )
                    dst = Kfm[:, m, ch * WS : (ch + 1) * WS]
                    if cfg.use_bk:
                        nc.scalar.activation(
                            out=dst, in_=ps, func=AF.Identity,
                            bias=bk_sb[:, m : m + 1], scale=1.0,
                        )
                    else:
                        nc.scalar.copy(dst, ps)
            # V (all tokens, token-major, augmented with a ones column per head)
            for t in range(S // KT):
                ps = pap.tile([128, D], F32, tag="ps")
                for k in range(DK):
                    nc.tensor.matmul(
                        out=ps,
                        lhsT=xT_sb[:, k, t * 128 : (t + 1) * 128],
                        rhs=wqkv_sb[:, k, 2 * D : 3 * D],
                        start=(k == 0),
                        stop=(k == DK - 1),
                    )
                vdst = Vaug[:, t, :].rearrange("p (h c) -> p h c", h=H)
                nc.vector.memset(vdst[:, :, HD : HD + 1], 1.0)
                nc.scalar.copy(
                    vdst[:, :, 0:HD],
                    ps.rearrange("p (h c) -> p h c", h=H),
                )

        pa_cm.__exit__(None, None, None)

        # ---------------- Phase B: attention --------------------------------
        with (
            tc.tile_pool(name="pb_sc", bufs=2, space="PSUM") as pbs,
            tc.tile_pool(name="pb_cx", bufs=2, space="PSUM") as pbc,
            tc.tile_pool(name="pb_es", bufs=3) as pbe,
            tc.tile_pool(name="pb_w", bufs=3) as pbw,
        ):
            for s in range(NSLOT):
                Es = E[s]
                for h in range(H):
                    m = h // HPT
                    off = (h % HPT) * HD
                    cps = pbc.tile([HD + 1, CHUNK], F32, tag="cps")
                    for qj in range(Es // 4):
                        sc = pbs.tile([128, 4 * CHUNK], F32, tag="sc")
                        for jj in range(4):
                            j = 4 * qj + jj
                            nc.tensor.matmul(
                                out=sc[:, jj * CHUNK : (jj + 1) * CHUNK],
                                lhsT=Kfm[off : off + HD, m, j * KT : (j + 1) * KT],
                                rhs=Qfm[off : off + HD, m, s * CHUNK : (s + 1) * CHUNK],
                                start=True,
                                stop=True,
                            )
                        es = pbe.tile([128, 4 * CHUNK], BF16, tag="es")
                        nc.scalar.activation(out=es, in_=sc, func=AF.Exp)
                        if qj == Es // 4 - 1:
                            nc.vector.tensor_mul(es, es, masks_sb[:, s, :])
                        for jj in range(4):
                            j = 4 * qj + jj
                            nc.tensor.matmul(
                                out=cps,
                                lhsT=Vaug[:, j, h * (HD + 1) : (h + 1) * (HD + 1)],
                                rhs=es[:, jj * CHUNK : (jj + 1) * CHUNK],
                                start=(j == 0),
                                stop=(j == Es - 1),
                            )
                    inv = pbw.tile([1, CHUNK], F32, tag="inv")
                    nc.vector.reciprocal(out=inv, in_=cps[HD : HD + 1, :])
                    invb = pbw.tile([HD, CHUNK], F32, tag="invb")
                    nc.gpsimd.partition_broadcast(invb, inv)
                    cdst = ctx_fm[off : off + HD, m, s * CHUNK : (s + 1) * CHUNK]
                    nc.vector.tensor_mul(cdst, cps[0:HD, :], invb)
                    if cfg.use_bv:
                        nc.scalar.add(cdst, cdst, bv_sb[off : off + HD, m : m + 1])

        pqkv_cm.__exit__(None, None, None)

        # -------- late-weight loads (DMAs overlap the attention phase: their
        # pool allocs only depend on phase-A pool releases) -------------------
        pc = top.enter_context(tc.tile_pool(name="pc", bufs=1))
        wo_sb = pc.tile([128, DK, D], BF16)
        nc.sync.dma_start(out=wo_sb, in_=wo_d.rearrange("(m p) c -> p m c", p=128))
        xown_sb = pc.tile([128, QT, D], F32)
        nc.sync.dma_start(out=xown_sb, in_=xown_d.rearrange("(t p) d -> p t d", p=128))
        pw1 = top.enter_context(tc.tile_pool(name="pw1", bufs=1))
        w1_sb = pw1.tile([128, DK, F], BF16)
        nc.sync.dma_start(out=w1_sb, in_=w1_d.rearrange("(m p) c -> p m c", p=128))

        # ---------------- Phase C: out-proj + LN1 + transpose ----------------
        pd = top.enter_context(tc.tile_pool(name="pd", bufs=1))
        xln1 = pd.tile([128, QT, D], F32)
        x1t = pd.tile([128, DK, NQ], BF16)

        pw2 = top.enter_context(tc.tile_pool(name="pw2", bufs=1))
        w2_sb = pw2.tile([128, FK, D], BF16)
        nc.sync.dma_start(out=w2_sb, in_=w2_d.rearrange("(f p) c -> p f c", p=128))

        def layer_norm_step(tt, g_b, bn_b, dst, work):
            # tt: [128, D] fp32 SBUF (modified in place is fine), dst: [128, D]
            stats = work.tile([128, nc.vector.BN_STATS_DIM], F32, tag="stats")
            nc.vector.bn_stats(out=stats, in_=tt)
            mv = work.tile([128, nc.vector.BN_AGGR_DIM], F32, tag="mv")
            nc.vector.bn_aggr(out=mv, in_=stats)
            sd = work.tile([128, 1], F32, tag="sd")
            nc.scalar.activation(out=sd, in_=mv[:, 1:2], func=AF.Sqrt, bias=eps_sb)
            rstd = work.tile([128, 1], F32, tag="rstd")
            nc.vector.reciprocal(out=rstd, in_=sd)
            nc.vector.tensor_scalar(
                out=dst, in0=tt, scalar1=mv[:, 0:1], scalar2=rstd,
                op0=ALU.subtract, op1=ALU.mult,
            )
            if g_b is not None:
                nc.vector.tensor_mul(dst, dst, g_b)
            if bn_b is not None:
                nc.vector.tensor_add(dst, dst, bn_b)

        with (
            tc.tile_pool(name="pc_ps", bufs=2, space="PSUM") as pcp,
            tc.tile_pool(name="pc_tp", bufs=2, space="PSUM") as pct,
            tc.tile_pool(name="pc_w", bufs=3) as pcw,
        ):
            for t in range(QT):
                ps = pcp.tile([128, D], F32, tag="ps")
                for m in range(DK):
                    nc.tensor.matmul(
                        out=ps,
                        lhsT=ctx_fm[:, m, t * 128 : (t + 1) * 128],
                        rhs=wo_sb[:, m, :],
                        start=(m == 0),
                        stop=(m == DK - 1),
                    )
                tt = pcw.tile([128, D], F32, tag="tt")
                nc.vector.tensor_add(tt, ps, xown_sb[:, t, :])
                layer_norm_step(tt, g1_b, bn1_b, xln1[:, t, :], pcw)
                for m in range(DK):
                    tp = pct.tile([128, 128], F32, tag="tp")
                    nc.tensor.transpose(
                        tp, xln1[:, t, m * 128 : (m + 1) * 128], ident
                    )
                    nc.scalar.copy(x1t[:, m, t * 128 : (t + 1) * 128], tp)

        # ---------------- Phase D: FFN + LN2 + store -------------------------
        with (
            tc.tile_pool(name="pf_h", bufs=2, space="PSUM") as pfh,
            tc.tile_pool(name="pf_y", bufs=1, space="PSUM") as pfy,
            tc.tile_pool(name="pf_hb", bufs=3) as pfhb,
            tc.tile_pool(name="pf_w", bufs=3) as pfw,
            tc.tile_pool(name="pf_o", bufs=2) as pfo,
        ):
            for ch in range(NQ // WQ):
                yps = [pfy.tile([128, D], F32, name=f"y{i}", tag=f"y{i}") for i in range(NSUB)]
                for f in range(FK):
                    hp = pfh.tile([128, WQ], F32, tag="hp")
                    for k in range(DK):
                        nc.tensor.matmul(
                            out=hp,
                            lhsT=w1_sb[:, k, f * 128 : (f + 1) * 128],
                            rhs=x1t[:, k, ch * WQ : (ch + 1) * WQ],
                            start=(k == 0),
                            stop=(k == DK - 1),
                        )
                    hb = pfhb.tile([128, WQ], BF16, tag="hb")
                    if cfg.use_b1:
                        nc.scalar.activation(
                            out=hb, in_=hp, func=AF.Relu,
                            bias=b1_sb[:, f : f + 1], scale=1.0,
                        )
                    else:
                        nc.scalar.activation(out=hb, in_=hp, func=AF.Relu)
                    for sub in range(NSUB):
                        nc.tensor.matmul(
                            out=yps[sub],
                            lhsT=hb[:, sub * 128 : (sub + 1) * 128],
                            rhs=w2_sb[:, f, :],
                            start=(f == 0),
                            stop=(f == FK - 1),
                        )
                for sub in range(NSUB):
                    t = ch * NSUB + sub
                    tt = pfw.tile([128, D], F32, tag="tt")
                    nc.vector.tensor_add(tt, yps[sub], xln1[:, t, :])
                    if cfg.use_b2:
                        nc.vector.tensor_add(tt, tt, b2_b)
                    ob = pfo.tile([128, D], F32, tag="ob")
                    layer_norm_step(tt, g2_b, bn2_b, ob, pfw)
                    nc.sync.dma_start(
                        out=out_d[t * 128 : (t + 1) * 128, :], in_=ob
                    )

    nc.compile()
    return nc


# ---------------------------------------------------------------------------
# host side
# ---------------------------------------------------------------------------

def build_masks(cfg: Cfg, half: int) -> np.ndarray:
    E = cfg.slot_E()
    qs_l = cfg.slot_qs(half)
    m = np.zeros((128, cfg.NSLOT, 4 * cfg.CHUNK), np.float32)
    k_loc = np.arange(128)[:, None]
    q_loc = np.arange(cfg.CHUNK)[None, :]
    for s, qs in enumerate(qs_l):
        jbase = E[s] - 4
        for jj in range(4):
            j = jbase + jj
            keep = (qs + q_loc) >= (j * cfg.KT + k_loc)
            m[:, s, jj * cfg.CHUNK : (jj + 1) * cfg.CHUNK] = keep
    return m.astype(ml_dtypes.bfloat16)


def host_prepare(inputs: dict, cfg: Cfg):
    """Returns (in_maps, own_idx_per_core)."""
    x = np.asarray(inputs["x"], np.float32)
    Wqkv = np.asarray(inputs["Wqkv"], np.float32)
    bqkv = np.asarray(inputs["bqkv"], np.float32)
    Wo = np.asarray(inputs["Wo"], np.float32)
    bo = np.asarray(inputs["bo"], np.float32)
    W1 = np.asarray(inputs["W1"], np.float32)
    b1 = np.asarray(inputs["b1"], np.float32)
    W2 = np.asarray(inputs["W2"], np.float32)
    b2 = np.asarray(inputs["b2"], np.float32)
    g1 = np.asarray(inputs["g1"], np.float32)
    bn1 = np.asarray(inputs["bn1"], np.float32)
    g2 = np.asarray(inputs["g2"], np.float32)
    bn2 = np.asarray(inputs["bn2"], np.float32)

    D = cfg.D
    scale = 1.0 / np.sqrt(np.float32(cfg.HD))
    wqkvT = np.concatenate(
        [
            np.ascontiguousarray(Wqkv[0:D].T) * scale,
            np.ascontiguousarray(Wqkv[D : 2 * D].T),
            np.ascontiguousarray(Wqkv[2 * D : 3 * D].T),
        ],
        axis=1,
    ).astype(np.float32)
    woT = np.ascontiguousarray(Wo.T)
    w1T = np.ascontiguousarray(W1.T)
    w2T = np.ascontiguousarray(W2.T)
    bq = bqkv[0:D] * scale
    bk = bqkv[D : 2 * D]
    bv = bqkv[2 * D : 3 * D]

    masks = [build_masks(cfg, half) for half in (0, 1)]

    in_maps = []
    own_idx_per_core = []
    for c in range(2 * cfg.B):
        b = c // 2
        half = c % 2
        own_idx = np.concatenate(
            [np.arange(qs, qs + cfg.CHUNK) for qs in cfg.slot_qs(half)]
        )
        own_idx_per_core.append(own_idx)
        xb = x[b]
        in_maps.append(
            {
                "xT": np.ascontiguousarray(xb.T).astype(ml_dtypes.bfloat16),
                "xqT": np.ascontiguousarray(xb[own_idx].T).astype(ml_dtypes.bfloat16),
                "xown": np.ascontiguousarray(xb[own_idx]) + bo[None, :],
                "wqkvT": wqkvT.astype(ml_dtypes.bfloat16),
                "woT": woT.astype(ml_dtypes.bfloat16),
                "w1T": w1T.astype(ml_dtypes.bfloat16),
                "w2T": w2T.astype(ml_dtypes.bfloat16),
                "bq": bq,
                "bk": bk,
                "bv": bv,
                "b1": b1,
                "b2": np.tile(b2[None, :], (128, 1)),
                "g1v": np.tile(g1[None, :], (128, 1)),
                "bn1v": np.tile(bn1[None, :], (128, 1)),
                "g2v": np.tile(g2[None, :], (128, 1)),
                "bn2v": np.tile(bn2[None, :], (128, 1)),
                "masks": masks[half],
            }
        )
    return in_maps, own_idx_per_core


def make_cfg(inputs: dict) -> Cfg:
    x = np.asarray(inputs["x"])
    B, S, D = x.shape
    F = np.asarray(inputs["W1"]).shape[0]
    bqkv = np.asarray(inputs["bqkv"], np.float32)
    cfg = Cfg(
        B=B, S=S, D=D, F=F,
        use_bq=bool(np.any(bqkv[0:D])),
        use_bk=bool(np.any(bqkv[D : 2 * D])),
        use_bv=bool(np.any(bqkv[2 * D : 3 * D])),
        use_b1=bool(np.any(np.asarray(inputs["b1"]))),
        use_b2=bool(np.any(np.asarray(inputs["b2"]))),
        use_g1=not bool(np.all(np.asarray(inputs["g1"]) == 1.0)),
        use_bn1=bool(np.any(np.asarray(inputs["bn1"]))),
        use_g2=not bool(np.all(np.asarray(inputs["g2"]) == 1.0)),
        use_bn2=bool(np.any(np.asarray(inputs["bn2"]))),
    )
    return cfg


_NC_CACHE: dict = {}

TRACE = False
LAST_RESULT = None


def kernel(**inputs) -> np.ndarray:
    global LAST_RESULT
    cfg = make_cfg(inputs)
    key = tuple(sorted(cfg.__dict__.items()))
    if key not in _NC_CACHE:
        _NC_CACHE[key] = build_nc(cfg)
    nc = _NC_CACHE[key]

    in_maps, own_idx_per_core = host_prepare(inputs, cfg)
    ncores = 2 * cfg.B
    res = run_bass_kernel_spmd(
        nc, in_maps, core_ids=list(range(ncores)), trace=TRACE
    )
    LAST_RESULT = res

    out = np.empty((cfg.B, cfg.S, cfg.D), np.float32)
    for c in range(ncores):
        out[c // 2, own_idx_per_core[c]] = res.results[c]["out"]
    return out
